# revision 22
# baseline (speedup 1.0000x reference)
"""Trainium2 Bass kernel for nn_EquivariantProductBasisBlock.

Math: per (n,c) with x = node_feats[n,c,:] in R^9, one-hot node_attrs:
  f[n,c,dt] = sum_k w3[n,k,c] * <U3sym[dt,:,k], mono3(x)>
            + sum_k w2[n,k,c] * <U2sym[dt,:,k], mono2(x)>
            + sum_k w1[n,k,c] * <U1[dt,:,k], x>
  out = concat_dt(f @ Wlin) / sqrt(C) + sc

The device builds the symmetric monomial basis itself from xT [9, F]
(the axon tunnel runs ~40 MB/s, so shipping host-staged monomials is
the bottleneck; shipping only xT cuts H2D ~25x). Per 512-col block
(4 node-slots x 128 channels, c-fastest):
  m2[45,F]   = (A2^T xT) * (B2^T xT)            (PE gathers + DVE mul)
  m3[165,F]  = (A3^T xT) * (P23^T m2)           (split 128+37 rows)
  G[124,F]   = CFa^T m3a + CFb3^T m3b + CF2^T m2  (PE, 3-way accum)
  T1         = G * WE32[elem]                    (DVE, c-broadcast AP)
  T1u        = (S1u^T xT) * WE1[elem]            (PE + DVE)
  f[4,F]     = R1^T T1 + R2^T T1u                (PE k-reduction)
Nodes are dealt to cores round-robin per element class so the
block->element map is identical on all 8 cores (SPMD-uniform).
Dispatch is a cached jax.jit(shard_map(bass_exec)) — U/W-derived
constants and zero output buffers live on device across calls; only
xT (fp16) is shipped per call and only f (fp16) is fetched back.
Host: scatter/gather permutation, final equivariant Linear + sc.
"""
import sys
import numpy as np

sys.path.insert(0, "/opt/trn_rl_repo")

N, C, I, E = 2048, 128, 9, 10
K3, K2, K1 = 23, 8, 3
NCORES = 8
FB = 256                  # free cols per block
SLOTS_PER_BLK = FB // C   # 4 node-slots per block

TRI3 = [(a, b, c) for a in range(I) for b in range(a, I) for c in range(b, I)]
TRI2 = [(a, b) for a in range(I) for b in range(a, I)]
M2IDX = {ab: r for r, ab in enumerate(TRI2)}
NM3, NM2 = len(TRI3), len(TRI2)           # 165, 45
NC3, NC2, NC1 = 4 * K3, 4 * K2, 4 * K1    # 92, 32, 12
NCOL = NC3 + NC2                          # 124
MAR = 128                                 # m3 rows in the A split
MBR3 = NM3 - MAR                          # 37
DT_LIST = [(0, 0), (1, 0), (1, 1), (1, 2)]

_RT = {}            # eb tuple -> runtime dict (nc, jitted dispatch, zeros)
_DEV_CONSTS = {}    # (consts md5) -> name -> sharded device array
_LAYOUT = {}        # (elem md5) -> layout dict
_HOST_CONSTS = {}   # (consts md5) -> np consts dict


def _build_consts(inputs):
    """Coefficient / weight matrices derived from the U/W input tensors."""
    U3s = [np.asarray(inputs["U3_0"]), np.asarray(inputs["U3_1"])]
    U2s = [np.asarray(inputs["U2_0"]), np.asarray(inputs["U2_1"])]
    U1s = [np.asarray(inputs["U1_0"]), np.asarray(inputs["U1_1"])]
    W3s = [np.asarray(inputs["W3_0"]), np.asarray(inputs["W3_1"])]
    W2s = [np.asarray(inputs["W2_0"]), np.asarray(inputs["W2_1"])]
    W1s = [np.asarray(inputs["W1_0"]), np.asarray(inputs["W1_1"])]

    # symmetrized U3/U2 -> CF [mono-row, (dt,k) col]
    CF3 = np.zeros((NM3, NCOL), np.float64)
    CF2 = np.zeros((NM2, NCOL), np.float64)
    tri3_idx = {m: r for r, m in enumerate(TRI3)}
    for di, (s, d) in enumerate(DT_LIST):
        u3 = np.zeros((NM3, K3), np.float64)
        u2 = np.zeros((NM2, K2), np.float64)
        U3 = np.asarray(U3s[s], np.float64)
        U2 = np.asarray(U2s[s], np.float64)
        for p in range(I):
            for q in range(I):
                u2[M2IDX[tuple(sorted((p, q)))]] += U2[d, p, q, :]
                for i in range(I):
                    u3[tri3_idx[tuple(sorted((p, q, i)))]] += U3[d, p, q, i, :]
        CF3[:, di * K3:(di + 1) * K3] = u3
        CF2[:, NC3 + di * K2:NC3 + (di + 1) * K2] = u2

    CFall = np.concatenate([CF3, CF2], axis=0)   # [210, 124]
    S1u = np.zeros((I, NC1), np.float32)         # U1 fold: U1X = S1u.T @ xT
    for di, (s, d) in enumerate(DT_LIST):
        S1u[:, di * K1:(di + 1) * K1] = U1s[s][d, :, :]

    R1 = np.zeros((NCOL, 4), np.float16)
    R2 = np.zeros((NC1, 4), np.float16)
    WE32 = np.zeros((NCOL, E, C), np.float32)
    WE1 = np.zeros((NC1, E, C), np.float32)
    for di, (s, d) in enumerate(DT_LIST):
        R1[di * K3:(di + 1) * K3, di] = 1.0
        R1[NC3 + di * K2:NC3 + (di + 1) * K2, di] = 1.0
        R2[di * K1:(di + 1) * K1, di] = 1.0
        WE32[di * K3:(di + 1) * K3] = W3s[s].transpose(1, 0, 2)
        WE32[NC3 + di * K2:NC3 + (di + 1) * K2] = W2s[s].transpose(1, 0, 2)
        WE1[di * K1:(di + 1) * K1] = W1s[s].transpose(1, 0, 2)

    # partition-gather matrices for on-device monomial construction
    A2 = np.zeros((I, NM2), np.float16)
    B2 = np.zeros((I, NM2), np.float16)
    for r, (a, b) in enumerate(TRI2):
        A2[a, r] = 1.0
        B2[b, r] = 1.0
    A3 = np.zeros((I, NM3), np.float16)
    P23 = np.zeros((NM2, NM3), np.float16)
    for r, (a, b, c) in enumerate(TRI3):
        A3[a, r] = 1.0
        P23[M2IDX[(b, c)], r] = 1.0

    return {
        "A2c": A2, "B2c": B2,
        "A3a": A3[:, :MAR].copy(), "A3b": A3[:, MAR:].copy(),
        "P23a": P23[:, :MAR].copy(), "P23b": P23[:, MAR:].copy(),
        "S1uc": S1u.astype(np.float16),
        "CFa": CFall[:MAR].astype(np.float16),
        "CFb3": CFall[MAR:NM3].astype(np.float16),
        "CF2": CFall[NM3:].astype(np.float16),
        "R1": R1, "R2": R2,
        "WE32": WE32.reshape(NCOL, E * C).astype(np.float16),
        "WE1": WE1.reshape(NC1, E * C).astype(np.float16),
    }


def _build_nc(eb, quant=False):
    """Bass program; eb = element id per block (same on all cores).

    quant=False: output f [4, FT] f16 + FM [4,1] f32 (running max of f^2).
    quant=True:  extra input QS [4,1] f32; output q = rne(f*QS) int8 + FM.
    """
    from concourse import bass, bacc, tile, mybir

    f32 = mybir.dt.float32
    f16 = mybir.dt.float16
    i8 = mybir.dt.int8
    NBLK = len(eb)
    FT = NBLK * FB

    nc = bacc.Bacc(None, target_bir_lowering=False, debug=False)
    xt_d = nc.declare_dram_parameter("XT", [I, FT], f16, isOutput=False)
    cshapes = {
        "A2c": ([I, NM2], f16), "B2c": ([I, NM2], f16),
        "A3a": ([I, MAR], f16), "A3b": ([I, MBR3], f16),
        "P23a": ([NM2, MAR], f16), "P23b": ([NM2, MBR3], f16),
        "S1uc": ([I, NC1], f16),
        "CFa": ([MAR, NCOL], f16), "CFb3": ([MBR3, NCOL], f16),
        "CF2": ([NM2, NCOL], f16),
        "R1": ([NCOL, 4], f16), "R2": ([NC1, 4], f16),
        "WE32": ([NCOL, E * C], f16), "WE1": ([NC1, E * C], f16),
    }
    cd = {k: nc.declare_dram_parameter(k, shp, dt, isOutput=False)
          for k, (shp, dt) in cshapes.items()}
    if quant:
        qs_d = nc.declare_dram_parameter("QS", [4, 1], f32, isOutput=False)
        f_d = nc.declare_dram_parameter("q", [4, FT], i8, isOutput=True)
    else:
        f_d = nc.declare_dram_parameter("f", [4, FT], f16, isOutput=True)

    with tile.TileContext(nc) as tc:
        with (
            tc.tile_pool(name="const", bufs=1) as cpool,
            tc.tile_pool(name="xin", bufs=2) as xpool,
            tc.tile_pool(name="work", bufs=3) as wpool,
            tc.tile_pool(name="stage", bufs=2) as spool,
            tc.tile_pool(name="psg", bufs=4, space=bass.MemorySpace.PSUM) as gp,
            tc.tile_pool(name="psG", bufs=2, space=bass.MemorySpace.PSUM) as pG,
            tc.tile_pool(name="psF", bufs=2, space=bass.MemorySpace.PSUM) as pF,
        ):
            ct = {}
            for k, (shp, dt) in cshapes.items():
                ct[k] = cpool.tile(shp, dt, tag=k, name=k)
                nc.sync.dma_start(out=ct[k][:], in_=cd[k][:])
            if quant:
                qs_t = cpool.tile([4, 1], f32, tag="QS", name="QS")
                nc.sync.dma_start(out=qs_t[:], in_=qs_d[:])

            XCHUNK = 4096 // FB  # blocks per x DMA chunk
            for b in range(NBLK):
                if b % XCHUNK == 0:
                    w = min(XCHUNK * FB, FT - b * FB)
                    xch = xpool.tile([I, XCHUNK * FB], f16, tag="xch")
                    nc.sync.dma_start(out=xch[:, :w],
                                      in_=xt_d[:, b * FB:b * FB + w])
                    fstage = spool.tile([4, XCHUNK * FB],
                                        i8 if quant else f16, tag="fst")
                o = (b % XCHUNK) * FB
                xt = xch[:, o:o + FB]
                e = eb[b]

                # m2 = (A2^T xT) * (B2^T xT)
                ga2 = gp.tile([NM2, FB], f32, tag="gat")
                nc.tensor.matmul(ga2[:], ct["A2c"][:], xt, start=True, stop=True)
                ca2 = wpool.tile([NM2, FB], f16, tag="ca2")
                nc.scalar.copy(ca2[:], ga2[:])
                gb2 = gp.tile([NM2, FB], f32, tag="gat")
                nc.tensor.matmul(gb2[:], ct["B2c"][:], xt, start=True, stop=True)
                m2 = wpool.tile([NM2, FB], f16, tag="m2")
                nc.vector.tensor_mul(m2[:], ca2[:], gb2[:])

                # m3 rows 0..127
                ga3a = gp.tile([MAR, FB], f32, tag="gat")
                nc.tensor.matmul(ga3a[:], ct["A3a"][:], xt, start=True, stop=True)
                ca3a = wpool.tile([MAR, FB], f16, tag="ca3a")
                nc.scalar.copy(ca3a[:], ga3a[:])
                gm2a = gp.tile([MAR, FB], f32, tag="gat")
                nc.tensor.matmul(gm2a[:], ct["P23a"][:], m2[:], start=True, stop=True)
                ma = wpool.tile([MAR, FB], f16, tag="ma")
                nc.vector.tensor_mul(ma[:], ca3a[:], gm2a[:])

                # m3 rows 128..164
                ga3b = gp.tile([MBR3, FB], f32, tag="gat")
                nc.tensor.matmul(ga3b[:], ct["A3b"][:], xt, start=True, stop=True)
                ca3b = wpool.tile([MBR3, FB], f16, tag="ca3b")
                nc.scalar.copy(ca3b[:], ga3b[:])
                gm2b = gp.tile([MBR3, FB], f32, tag="gat")
                nc.tensor.matmul(gm2b[:], ct["P23b"][:], m2[:], start=True, stop=True)
                mb = wpool.tile([MBR3, FB], f16, tag="mb")
                nc.vector.tensor_mul(mb[:], ca3b[:], gm2b[:])

                # T1u = (S1u^T xT) * WE1[elem]
                ux = gp.tile([NC1, FB], f32, tag="gat")
                nc.tensor.matmul(ux[:], ct["S1uc"][:], xt, start=True, stop=True)
                we1 = ct["WE1"][:, e * C:(e + 1) * C]
                we1b = we1.unsqueeze(1).broadcast_to([NC1, SLOTS_PER_BLK, C])
                t1u = wpool.tile([NC1, SLOTS_PER_BLK, C], f16, tag="t1u")
                nc.vector.tensor_mul(
                    t1u[:],
                    ux[:].rearrange("p (n c) -> p n c", n=SLOTS_PER_BLK), we1b)

                # G = CFa^T m3a + CFb3^T m3b + CF2^T m2
                g = pG.tile([NCOL, FB], f32, tag="g")
                nc.tensor.matmul(g[:], ct["CFa"][:], ma[:], start=True, stop=False)
                nc.tensor.matmul(g[:], ct["CFb3"][:], mb[:], start=False, stop=False)
                nc.tensor.matmul(g[:], ct["CF2"][:], m2[:], start=False, stop=True)

                we = ct["WE32"][:, e * C:(e + 1) * C]
                web = we.unsqueeze(1).broadcast_to([NCOL, SLOTS_PER_BLK, C])
                t1 = wpool.tile([NCOL, SLOTS_PER_BLK, C], f16, tag="t1")
                nc.vector.tensor_mul(
                    t1[:],
                    g[:].rearrange("p (n c) -> p n c", n=SLOTS_PER_BLK), web)

                f_ps = pF.tile([4, FB], f32, tag="f")
                nc.tensor.matmul(f_ps[:], ct["R1"][:],
                                 t1[:].rearrange("p n c -> p (n c)"),
                                 start=True, stop=False)
                nc.tensor.matmul(f_ps[:], ct["R2"][:],
                                 t1u[:].rearrange("p n c -> p (n c)"),
                                 start=False, stop=True)
                if quant:
                    # q = rne(f * QS) with int8 saturation; scale headroom
                    # keeps legit |q| < 127 so the host can detect clipping
                    nc.vector.tensor_scalar(
                        out=fstage[:, o:o + FB], in0=f_ps[:],
                        scalar1=qs_t[:], scalar2=None,
                        op0=mybir.AluOpType.mult)
                else:
                    nc.scalar.copy(fstage[:, o:o + FB], f_ps[:])
                if b % XCHUNK == XCHUNK - 1 or b == NBLK - 1:
                    lo = (b // XCHUNK) * XCHUNK
                    w = (b - lo + 1) * FB
                    nc.sync.dma_start(out=f_d[:, lo * FB:lo * FB + w],
                                      in_=fstage[:, :w])

    nc.compile()
    return nc


def _layout(elem):
    """Node -> (core, slot) dealing; identical block->element map per core."""
    key = elem.tobytes()
    if key in _LAYOUT:
        return _LAYOUT[key]
    count = np.bincount(elem, minlength=E)
    spe = [int(np.ceil(c / NCORES)) if c else 0 for c in count]
    blocks_e = [int(np.ceil(s / SLOTS_PER_BLK)) for s in spe]
    eb = []
    base_slot = []
    for e in range(E):
        base_slot.append(len(eb) * SLOTS_PER_BLK)
        eb.extend([e] * blocks_e[e])
    NBLK = len(eb)
    NSLOT = NBLK * SLOTS_PER_BLK
    order = np.argsort(elem, kind="stable")
    core_of = np.empty(N, np.int64)
    slot_of = np.empty(N, np.int64)
    pos = 0
    for e in range(E):
        idx = order[pos:pos + count[e]]
        pos += count[e]
        j = np.arange(count[e])
        core_of[idx] = j % NCORES
        slot_of[idx] = base_slot[e] + j // NCORES
    # gather index: gidx[core, slot] = node id, or N for padding
    gidx = np.full((NCORES, NSLOT), N, np.int64)
    gidx[core_of, slot_of] = np.arange(N)
    lay = {"eb": tuple(eb), "NBLK": NBLK, "NSLOT": NSLOT, "FT": NBLK * FB,
           "core_of": core_of, "slot_of": slot_of, "gidx": gidx}
    _LAYOUT[key] = lay
    return lay


def _get_rt(eb, quant=False):
    """Compile the Bass program and build the cached jitted dispatch."""
    key = (tuple(eb), quant)
    if key in _RT:
        return _RT[key]
    import jax
    from jax.sharding import Mesh, PartitionSpec, NamedSharding
    from jax.experimental.shard_map import shard_map
    from concourse import mybir
    from concourse.bass2jax import (_bass_exec_p, install_neuronx_cc_hook,
                                    partition_id_tensor)

    install_neuronx_cc_hook()
    nc = _build_nc(list(key[0]), quant=quant)

    partition_name = nc.partition_id_tensor.name if nc.partition_id_tensor else None
    in_names, out_names, out_avals, zero_shapes = [], [], [], []
    for alloc in nc.m.functions[0].allocations:
        if not isinstance(alloc, mybir.MemoryLocationSet):
            continue
        name = alloc.memorylocations[0].name
        if alloc.kind == "ExternalInput":
            if name != partition_name:
                in_names.append(name)
        elif alloc.kind == "ExternalOutput":
            out_names.append(name)
            shape = tuple(alloc.tensor_shape)
            dtype = mybir.dt.np(alloc.dtype)
            out_avals.append(jax.core.ShapedArray(shape, dtype))
            zero_shapes.append((shape, dtype))
    n_params = len(in_names)
    in_names_full = in_names + out_names + (
        [partition_name] if partition_name else [])

    def _body(*args):
        operands = list(args)
        if partition_name is not None:
            operands.append(partition_id_tensor())
        outs = _bass_exec_p.bind(
            *operands, out_avals=tuple(out_avals),
            in_names=tuple(in_names_full), out_names=tuple(out_names),
            lowering_input_output_aliases=(), sim_require_finite=True,
            sim_require_nnan=True, nc=nc)
        return tuple(outs)

    devices = jax.devices()[:NCORES]
    mesh = Mesh(np.asarray(devices), ("core",))
    nin = n_params + len(out_names)
    sh = NamedSharding(mesh, PartitionSpec("core"))

    def _make_jit():
        return jax.jit(
            shard_map(_body, mesh=mesh,
                      in_specs=(PartitionSpec("core"),) * nin,
                      out_specs=(PartitionSpec("core"),) * len(out_names),
                      check_rep=False),
            keep_unused=True)

    # abstract avals for AOT lowering (global shapes, sharded on axis 0)
    name2shape = {}
    for alloc in nc.m.functions[0].allocations:
        if isinstance(alloc, mybir.MemoryLocationSet) and alloc.tensor_shape:
            from concourse import mybir as _mb
            name2shape[alloc.memorylocations[0].name] = (
                tuple(alloc.tensor_shape), _mb.dt.np(alloc.dtype))
    structs = []
    for nm in in_names + out_names:
        shp, dt = name2shape[nm]
        structs.append(jax.ShapeDtypeStruct(
            (NCORES * shp[0],) + tuple(shp[1:]), dt, sharding=sh))
    try:
        from concourse.bass2jax import fast_dispatch_compile
        sharded = fast_dispatch_compile(
            lambda: _make_jit().lower(*structs).compile())
    except Exception:
        sharded = _make_jit()
    # zero output operands live on device; not donated, so reusable forever
    zeros = [jax.device_put(
        np.zeros((NCORES * s[0], *s[1:]), dt), sh) for s, dt in zero_shapes]
    rt = {"nc": nc, "sharded": sharded, "in_names": in_names,
          "out_names": out_names, "zeros": zeros, "sh": sh, "mesh": mesh}
    _RT[key] = rt
    return rt


def _get_dev_consts(inputs, rt):
    import jax
    import hashlib
    h = hashlib.md5()
    for k in ("U3_0", "U2_0", "U1_0", "W3_0", "W2_0", "W1_0",
              "U3_1", "U2_1", "U1_1", "W3_1", "W2_1", "W1_1"):
        h.update(np.ascontiguousarray(np.asarray(inputs[k])).tobytes())
    key = h.hexdigest()
    if key in _DEV_CONSTS:
        return _DEV_CONSTS[key]
    consts = _build_consts(inputs)
    dev = {k: jax.device_put(np.tile(v, (NCORES,) + (1,) * (v.ndim - 1)),
                             rt["sh"]) for k, v in consts.items()}
    for z in dev.values():
        z.block_until_ready()
    _DEV_CONSTS[key] = dev
    return dev


_QSTATE = {"scale": None, "qs_dev": None}
QUANT_HEADROOM = 1.1


def _dispatch(lay, dev, xt16, quant, jx):
    rt = _get_rt(lay["eb"], quant=quant)
    qargs = {"XT": xt16}
    if quant:
        qargs["QS"] = _QSTATE["qs_dev"]
    args = [qargs.get(nm, dev.get(nm)) for nm in rt["in_names"]]
    out = rt["sharded"](*args, *rt["zeros"])
    names = rt["out_names"]
    return {nm: out[i] for i, nm in enumerate(names)}


def _run(inputs):
    import os
    import jax as jx
    x = np.asarray(inputs["node_feats"], np.float32)
    sc = np.asarray(inputs["sc"], np.float32)
    y = np.asarray(inputs["node_attrs"], np.float32)
    Wlin0 = np.asarray(inputs["Wlin0"], np.float32)
    Wlin1 = np.asarray(inputs["Wlin1"], np.float32)

    elem = np.argmax(y, axis=1)
    lay = _layout(elem)
    NSLOT = lay["NSLOT"]
    rt0 = _get_rt(lay["eb"], quant=False)
    dev = _get_dev_consts(inputs, rt0)

    # one fused scatter: xt4d[core, i, slot, c] = x[node, c, i]; pad slots
    # stay zero from allocation (node slots are overwritten every call)
    xt4d = lay.setdefault(
        "xt4d", np.zeros((NCORES, I, NSLOT, C), np.float16))
    xt4d[lay["core_of"], :, lay["slot_of"]] = x.swapaxes(1, 2)
    xt16 = xt4d.reshape(NCORES * I, lay["FT"])

    quant = _QSTATE["scale"] is not None and not os.environ.get("BASS_NO_QUANT")
    scale = _QSTATE["scale"]
    if quant:
        out = _dispatch(lay, dev, xt16, True, jx)
        q_np = np.asarray(out["q"])
        if np.abs(q_np).max() >= 127:             # clipped: redo unquantized
            quant = False
    if not quant:
        out = _dispatch(lay, dev, xt16, False, jx)
        f_np = np.asarray(out["f"])
        fmax = np.abs(f_np).max(axis=1).reshape(NCORES, 4).max(axis=0)
        newscale = 127.0 / np.maximum(
            fmax.astype(np.float64) * QUANT_HEADROOM, 1e-30)
        _QSTATE["scale"] = newscale
        _QSTATE["qs_dev"] = jx.device_put(
            np.tile(newscale.reshape(4, 1).astype(np.float32), (NCORES, 1)),
            rt0["sh"])
        fall = f_np.reshape(NCORES, 4, NSLOT, C)
        deq = None
    else:
        fall = q_np.reshape(NCORES, 4, NSLOT, C)
        deq = (1.0 / scale).astype(np.float32)

    # fg[n, d, c] = f value for node n, irrep-component d, channel c
    fg = fall[lay["core_of"], :, lay["slot_of"], :].astype(np.float32)

    inv = np.float32(1.0 / np.sqrt(C))
    out_np = np.empty((N, C * 4), np.float32)
    np.matmul(fg[:, 0, :], Wlin0, out=out_np[:, :C])
    z = (fg[:, 1:, :].reshape(N * 3, C) @ Wlin1).reshape(N, 3, C)
    if deq is None:
        out_np[:, :C] *= inv
        out_np[:, C:] = z.transpose(0, 2, 1).reshape(N, 3 * C)
        out_np[:, C:] *= inv
    else:
        out_np[:, :C] *= inv * deq[0]
        z *= (inv * deq[1:]).reshape(1, 3, 1)
        out_np[:, C:] = z.transpose(0, 2, 1).reshape(N, 3 * C)
    out_np += sc
    return out_np


def kernel(**inputs):
    res = _run(inputs)
    import os
    nrep = int(os.environ.get("KERNEL_TIME_RUNS", "0"))
    if nrep:
        import time
        times = []
        for _ in range(nrep):
            t0 = time.perf_counter()
            _run(inputs)
            times.append(time.perf_counter() - t0)
        globals()["LAST_TIMES"] = times
    return res


# revision 24
# speedup vs baseline: 1.0158x; 1.0158x over previous
"""Trainium2 Bass kernel for nn_EquivariantProductBasisBlock.

Math: per (n,c) with x = node_feats[n,c,:] in R^9, one-hot node_attrs:
  f[n,c,dt] = sum_k w3[n,k,c] * <U3sym[dt,:,k], mono3(x)>
            + sum_k w2[n,k,c] * <U2sym[dt,:,k], mono2(x)>
            + sum_k w1[n,k,c] * <U1[dt,:,k], x>
  out = concat_dt(f @ Wlin) / sqrt(C) + sc

The device builds the symmetric monomial basis itself from xT [9, F]
(the axon tunnel runs ~40 MB/s, so shipping host-staged monomials is
the bottleneck; shipping only xT cuts H2D ~25x). Per 512-col block
(4 node-slots x 128 channels, c-fastest):
  m2[45,F]   = (A2^T xT) * (B2^T xT)            (PE gathers + DVE mul)
  m3[165,F]  = (A3^T xT) * (P23^T m2)           (split 128+37 rows)
  G[124,F]   = CFa^T m3a + CFb3^T m3b + CF2^T m2  (PE, 3-way accum)
  T1         = G * WE32[elem]                    (DVE, c-broadcast AP)
  T1u        = (S1u^T xT) * WE1[elem]            (PE + DVE)
  f[4,F]     = R1^T T1 + R2^T T1u                (PE k-reduction)
Nodes are dealt to cores round-robin per element class so the
block->element map is identical on all 8 cores (SPMD-uniform).
Dispatch is a cached jax.jit(shard_map(bass_exec)) — U/W-derived
constants and zero output buffers live on device across calls; only
xT (fp16) is shipped per call and only f (fp16) is fetched back.
Host: scatter/gather permutation, final equivariant Linear + sc.
"""
import sys
import numpy as np

sys.path.insert(0, "/opt/trn_rl_repo")

N, C, I, E = 2048, 128, 9, 10
K3, K2, K1 = 23, 8, 3
NCORES = 8
FB = 256                  # free cols per block
SLOTS_PER_BLK = FB // C   # 4 node-slots per block

TRI3 = [(a, b, c) for a in range(I) for b in range(a, I) for c in range(b, I)]
TRI2 = [(a, b) for a in range(I) for b in range(a, I)]
M2IDX = {ab: r for r, ab in enumerate(TRI2)}
NM3, NM2 = len(TRI3), len(TRI2)           # 165, 45
NC3, NC2, NC1 = 4 * K3, 4 * K2, 4 * K1    # 92, 32, 12
NCOL = NC3 + NC2                          # 124
MAR = 128                                 # m3 rows in the A split
MBR3 = NM3 - MAR                          # 37
DT_LIST = [(0, 0), (1, 0), (1, 1), (1, 2)]

_RT = {}            # eb tuple -> runtime dict (nc, jitted dispatch, zeros)
_DEV_CONSTS = {}    # (consts md5) -> name -> sharded device array
_LAYOUT = {}        # (elem md5) -> layout dict


def _build_consts(inputs):
    """Coefficient / weight matrices derived from the U/W input tensors."""
    U3s = [np.asarray(inputs["U3_0"]), np.asarray(inputs["U3_1"])]
    U2s = [np.asarray(inputs["U2_0"]), np.asarray(inputs["U2_1"])]
    U1s = [np.asarray(inputs["U1_0"]), np.asarray(inputs["U1_1"])]
    W3s = [np.asarray(inputs["W3_0"]), np.asarray(inputs["W3_1"])]
    W2s = [np.asarray(inputs["W2_0"]), np.asarray(inputs["W2_1"])]
    W1s = [np.asarray(inputs["W1_0"]), np.asarray(inputs["W1_1"])]

    # symmetrized U3/U2 -> CF [mono-row, (dt,k) col]
    CF3 = np.zeros((NM3, NCOL), np.float64)
    CF2 = np.zeros((NM2, NCOL), np.float64)
    tri3_idx = {m: r for r, m in enumerate(TRI3)}
    for di, (s, d) in enumerate(DT_LIST):
        u3 = np.zeros((NM3, K3), np.float64)
        u2 = np.zeros((NM2, K2), np.float64)
        U3 = np.asarray(U3s[s], np.float64)
        U2 = np.asarray(U2s[s], np.float64)
        for p in range(I):
            for q in range(I):
                u2[M2IDX[tuple(sorted((p, q)))]] += U2[d, p, q, :]
                for i in range(I):
                    u3[tri3_idx[tuple(sorted((p, q, i)))]] += U3[d, p, q, i, :]
        CF3[:, di * K3:(di + 1) * K3] = u3
        CF2[:, NC3 + di * K2:NC3 + (di + 1) * K2] = u2

    CFall = np.concatenate([CF3, CF2], axis=0)   # [210, 124]
    S1u = np.zeros((I, NC1), np.float32)         # U1 fold: U1X = S1u.T @ xT
    for di, (s, d) in enumerate(DT_LIST):
        S1u[:, di * K1:(di + 1) * K1] = U1s[s][d, :, :]

    R1 = np.zeros((NCOL, 4), np.float16)
    R2 = np.zeros((NC1, 4), np.float16)
    WE32 = np.zeros((NCOL, E, C), np.float32)
    WE1 = np.zeros((NC1, E, C), np.float32)
    for di, (s, d) in enumerate(DT_LIST):
        R1[di * K3:(di + 1) * K3, di] = 1.0
        R1[NC3 + di * K2:NC3 + (di + 1) * K2, di] = 1.0
        R2[di * K1:(di + 1) * K1, di] = 1.0
        WE32[di * K3:(di + 1) * K3] = W3s[s].transpose(1, 0, 2)
        WE32[NC3 + di * K2:NC3 + (di + 1) * K2] = W2s[s].transpose(1, 0, 2)
        WE1[di * K1:(di + 1) * K1] = W1s[s].transpose(1, 0, 2)

    # partition-gather matrices for on-device monomial construction
    A2 = np.zeros((I, NM2), np.float16)
    B2 = np.zeros((I, NM2), np.float16)
    for r, (a, b) in enumerate(TRI2):
        A2[a, r] = 1.0
        B2[b, r] = 1.0
    A3 = np.zeros((I, NM3), np.float16)
    P23 = np.zeros((NM2, NM3), np.float16)
    for r, (a, b, c) in enumerate(TRI3):
        A3[a, r] = 1.0
        P23[M2IDX[(b, c)], r] = 1.0

    return {
        "A2c": A2, "B2c": B2,
        "A3a": A3[:, :MAR].copy(), "A3b": A3[:, MAR:].copy(),
        "P23a": P23[:, :MAR].copy(), "P23b": P23[:, MAR:].copy(),
        "S1uc": S1u.astype(np.float16),
        "CFa": CFall[:MAR].astype(np.float16),
        "CFb3": CFall[MAR:NM3].astype(np.float16),
        "CF2": CFall[NM3:].astype(np.float16),
        "R1": R1, "R2": R2,
        "WE32": WE32.reshape(NCOL, E * C).astype(np.float16),
        "WE1": WE1.reshape(NC1, E * C).astype(np.float16),
    }


def _build_nc(eb):
    """Bass program; eb = element id per block (same on all cores)."""
    from concourse import bass, bacc, tile, mybir

    f32 = mybir.dt.float32
    f16 = mybir.dt.float16
    NBLK = len(eb)
    FT = NBLK * FB

    nc = bacc.Bacc(None, target_bir_lowering=False, debug=False)
    xt_d = nc.declare_dram_parameter("XT", [I, FT], f16, isOutput=False)
    cshapes = {
        "A2c": ([I, NM2], f16), "B2c": ([I, NM2], f16),
        "A3a": ([I, MAR], f16), "A3b": ([I, MBR3], f16),
        "P23a": ([NM2, MAR], f16), "P23b": ([NM2, MBR3], f16),
        "S1uc": ([I, NC1], f16),
        "CFa": ([MAR, NCOL], f16), "CFb3": ([MBR3, NCOL], f16),
        "CF2": ([NM2, NCOL], f16),
        "R1": ([NCOL, 4], f16), "R2": ([NC1, 4], f16),
        "WE32": ([NCOL, E * C], f16), "WE1": ([NC1, E * C], f16),
    }
    cd = {k: nc.declare_dram_parameter(k, shp, dt, isOutput=False)
          for k, (shp, dt) in cshapes.items()}
    f_d = nc.declare_dram_parameter("f", [4, FT], f16, isOutput=True)

    with tile.TileContext(nc) as tc:
        with (
            tc.tile_pool(name="const", bufs=1) as cpool,
            tc.tile_pool(name="xin", bufs=2) as xpool,
            tc.tile_pool(name="work", bufs=3) as wpool,
            tc.tile_pool(name="stage", bufs=2) as spool,
            tc.tile_pool(name="psg", bufs=4, space=bass.MemorySpace.PSUM) as gp,
            tc.tile_pool(name="psG", bufs=2, space=bass.MemorySpace.PSUM) as pG,
            tc.tile_pool(name="psF", bufs=2, space=bass.MemorySpace.PSUM) as pF,
        ):
            ct = {}
            for k, (shp, dt) in cshapes.items():
                ct[k] = cpool.tile(shp, dt, tag=k, name=k)
                nc.sync.dma_start(out=ct[k][:], in_=cd[k][:])

            XCHUNK = 4096 // FB  # blocks per x DMA chunk
            for b in range(NBLK):
                if b % XCHUNK == 0:
                    w = min(XCHUNK * FB, FT - b * FB)
                    xch = xpool.tile([I, XCHUNK * FB], f16, tag="xch")
                    nc.sync.dma_start(out=xch[:, :w],
                                      in_=xt_d[:, b * FB:b * FB + w])
                    fstage = spool.tile([4, XCHUNK * FB], f16, tag="fst")
                o = (b % XCHUNK) * FB
                xt = xch[:, o:o + FB]
                e = eb[b]

                # m2 = (A2^T xT) * (B2^T xT)
                ga2 = gp.tile([NM2, FB], f32, tag="gat")
                nc.tensor.matmul(ga2[:], ct["A2c"][:], xt, start=True, stop=True)
                ca2 = wpool.tile([NM2, FB], f16, tag="ca2")
                nc.scalar.copy(ca2[:], ga2[:])
                gb2 = gp.tile([NM2, FB], f32, tag="gat")
                nc.tensor.matmul(gb2[:], ct["B2c"][:], xt, start=True, stop=True)
                m2 = wpool.tile([NM2, FB], f16, tag="m2")
                nc.vector.tensor_mul(m2[:], ca2[:], gb2[:])

                # m3 rows 0..127
                ga3a = gp.tile([MAR, FB], f32, tag="gat")
                nc.tensor.matmul(ga3a[:], ct["A3a"][:], xt, start=True, stop=True)
                ca3a = wpool.tile([MAR, FB], f16, tag="ca3a")
                nc.scalar.copy(ca3a[:], ga3a[:])
                gm2a = gp.tile([MAR, FB], f32, tag="gat")
                nc.tensor.matmul(gm2a[:], ct["P23a"][:], m2[:], start=True, stop=True)
                ma = wpool.tile([MAR, FB], f16, tag="ma")
                nc.vector.tensor_mul(ma[:], ca3a[:], gm2a[:])

                # m3 rows 128..164
                ga3b = gp.tile([MBR3, FB], f32, tag="gat")
                nc.tensor.matmul(ga3b[:], ct["A3b"][:], xt, start=True, stop=True)
                ca3b = wpool.tile([MBR3, FB], f16, tag="ca3b")
                nc.scalar.copy(ca3b[:], ga3b[:])
                gm2b = gp.tile([MBR3, FB], f32, tag="gat")
                nc.tensor.matmul(gm2b[:], ct["P23b"][:], m2[:], start=True, stop=True)
                mb = wpool.tile([MBR3, FB], f16, tag="mb")
                nc.vector.tensor_mul(mb[:], ca3b[:], gm2b[:])

                # T1u = (S1u^T xT) * WE1[elem]
                ux = gp.tile([NC1, FB], f32, tag="gat")
                nc.tensor.matmul(ux[:], ct["S1uc"][:], xt, start=True, stop=True)
                we1 = ct["WE1"][:, e * C:(e + 1) * C]
                we1b = we1.unsqueeze(1).broadcast_to([NC1, SLOTS_PER_BLK, C])
                t1u = wpool.tile([NC1, SLOTS_PER_BLK, C], f16, tag="t1u")
                nc.vector.tensor_mul(
                    t1u[:],
                    ux[:].rearrange("p (n c) -> p n c", n=SLOTS_PER_BLK), we1b)

                # G = CFa^T m3a + CFb3^T m3b + CF2^T m2
                g = pG.tile([NCOL, FB], f32, tag="g")
                nc.tensor.matmul(g[:], ct["CFa"][:], ma[:], start=True, stop=False)
                nc.tensor.matmul(g[:], ct["CFb3"][:], mb[:], start=False, stop=False)
                nc.tensor.matmul(g[:], ct["CF2"][:], m2[:], start=False, stop=True)

                we = ct["WE32"][:, e * C:(e + 1) * C]
                web = we.unsqueeze(1).broadcast_to([NCOL, SLOTS_PER_BLK, C])
                t1 = wpool.tile([NCOL, SLOTS_PER_BLK, C], f16, tag="t1")
                nc.vector.tensor_mul(
                    t1[:],
                    g[:].rearrange("p (n c) -> p n c", n=SLOTS_PER_BLK), web)

                f_ps = pF.tile([4, FB], f32, tag="f")
                nc.tensor.matmul(f_ps[:], ct["R1"][:],
                                 t1[:].rearrange("p n c -> p (n c)"),
                                 start=True, stop=False)
                nc.tensor.matmul(f_ps[:], ct["R2"][:],
                                 t1u[:].rearrange("p n c -> p (n c)"),
                                 start=False, stop=True)
                nc.scalar.copy(fstage[:, o:o + FB], f_ps[:])
                if b % XCHUNK == XCHUNK - 1 or b == NBLK - 1:
                    lo = (b // XCHUNK) * XCHUNK
                    w = (b - lo + 1) * FB
                    nc.sync.dma_start(out=f_d[:, lo * FB:lo * FB + w],
                                      in_=fstage[:, :w])

    nc.compile()
    return nc


def _layout(elem):
    """Node -> (core, slot) dealing; identical block->element map per core."""
    key = elem.tobytes()
    if key in _LAYOUT:
        return _LAYOUT[key]
    count = np.bincount(elem, minlength=E)
    spe = [int(np.ceil(c / NCORES)) if c else 0 for c in count]
    blocks_e = [int(np.ceil(s / SLOTS_PER_BLK)) for s in spe]
    eb = []
    base_slot = []
    for e in range(E):
        base_slot.append(len(eb) * SLOTS_PER_BLK)
        eb.extend([e] * blocks_e[e])
    NBLK = len(eb)
    NSLOT = NBLK * SLOTS_PER_BLK
    order = np.argsort(elem, kind="stable")
    core_of = np.empty(N, np.int64)
    slot_of = np.empty(N, np.int64)
    pos = 0
    for e in range(E):
        idx = order[pos:pos + count[e]]
        pos += count[e]
        j = np.arange(count[e])
        core_of[idx] = j % NCORES
        slot_of[idx] = base_slot[e] + j // NCORES
    # gather index: gidx[core, slot] = node id, or N for padding
    gidx = np.full((NCORES, NSLOT), N, np.int64)
    gidx[core_of, slot_of] = np.arange(N)
    lay = {"eb": tuple(eb), "NBLK": NBLK, "NSLOT": NSLOT, "FT": NBLK * FB,
           "core_of": core_of, "slot_of": slot_of, "gidx": gidx}
    _LAYOUT[key] = lay
    return lay


def _get_rt(eb):
    """Compile the Bass program and build the cached jitted dispatch."""
    key = tuple(eb)
    if key in _RT:
        return _RT[key]
    import jax
    from jax.sharding import Mesh, PartitionSpec, NamedSharding
    from jax.experimental.shard_map import shard_map
    from concourse import mybir
    from concourse.bass2jax import (_bass_exec_p, install_neuronx_cc_hook,
                                    partition_id_tensor)

    install_neuronx_cc_hook()
    nc = _build_nc(list(key))

    partition_name = nc.partition_id_tensor.name if nc.partition_id_tensor else None
    in_names, out_names, out_avals, zero_shapes = [], [], [], []
    for alloc in nc.m.functions[0].allocations:
        if not isinstance(alloc, mybir.MemoryLocationSet):
            continue
        name = alloc.memorylocations[0].name
        if alloc.kind == "ExternalInput":
            if name != partition_name:
                in_names.append(name)
        elif alloc.kind == "ExternalOutput":
            out_names.append(name)
            shape = tuple(alloc.tensor_shape)
            dtype = mybir.dt.np(alloc.dtype)
            out_avals.append(jax.core.ShapedArray(shape, dtype))
            zero_shapes.append((shape, dtype))
    n_params = len(in_names)
    in_names_full = in_names + out_names + (
        [partition_name] if partition_name else [])

    def _body(*args):
        operands = list(args)
        if partition_name is not None:
            operands.append(partition_id_tensor())
        outs = _bass_exec_p.bind(
            *operands, out_avals=tuple(out_avals),
            in_names=tuple(in_names_full), out_names=tuple(out_names),
            lowering_input_output_aliases=(), sim_require_finite=True,
            sim_require_nnan=True, nc=nc)
        return tuple(outs)

    devices = jax.devices()[:NCORES]
    mesh = Mesh(np.asarray(devices), ("core",))
    nin = n_params + len(out_names)
    sh = NamedSharding(mesh, PartitionSpec("core"))

    def _make_jit():
        return jax.jit(
            shard_map(_body, mesh=mesh,
                      in_specs=(PartitionSpec("core"),) * nin,
                      out_specs=(PartitionSpec("core"),) * len(out_names),
                      check_rep=False),
            keep_unused=True)

    # abstract avals for AOT lowering (global shapes, sharded on axis 0)
    name2shape = {}
    for alloc in nc.m.functions[0].allocations:
        if isinstance(alloc, mybir.MemoryLocationSet) and alloc.tensor_shape:
            from concourse import mybir as _mb
            name2shape[alloc.memorylocations[0].name] = (
                tuple(alloc.tensor_shape), _mb.dt.np(alloc.dtype))
    structs = []
    for nm in in_names + out_names:
        shp, dt = name2shape[nm]
        structs.append(jax.ShapeDtypeStruct(
            (NCORES * shp[0],) + tuple(shp[1:]), dt, sharding=sh))
    try:
        from concourse.bass2jax import fast_dispatch_compile
        sharded = fast_dispatch_compile(
            lambda: _make_jit().lower(*structs).compile())
    except Exception:
        sharded = _make_jit()
    # zero output operands live on device; not donated, so reusable forever
    zeros = [jax.device_put(
        np.zeros((NCORES * s[0], *s[1:]), dt), sh) for s, dt in zero_shapes]
    rt = {"nc": nc, "sharded": sharded, "in_names": in_names,
          "out_names": out_names, "zeros": zeros, "sh": sh, "mesh": mesh}
    _RT[key] = rt
    return rt


def _get_dev_consts(inputs, rt):
    import jax
    import hashlib
    h = hashlib.md5()
    for k in ("U3_0", "U2_0", "U1_0", "W3_0", "W2_0", "W1_0",
              "U3_1", "U2_1", "U1_1", "W3_1", "W2_1", "W1_1"):
        h.update(np.ascontiguousarray(np.asarray(inputs[k])).tobytes())
    key = h.hexdigest()
    if key in _DEV_CONSTS:
        return _DEV_CONSTS[key]
    consts = _build_consts(inputs)
    dev = {k: jax.device_put(np.tile(v, (NCORES,) + (1,) * (v.ndim - 1)),
                             rt["sh"]) for k, v in consts.items()}
    for z in dev.values():
        z.block_until_ready()
    _DEV_CONSTS[key] = dev
    return dev


def _run(inputs):
    x = np.asarray(inputs["node_feats"], np.float32)
    sc = np.asarray(inputs["sc"], np.float32)
    y = np.asarray(inputs["node_attrs"], np.float32)
    Wlin0 = np.asarray(inputs["Wlin0"], np.float32)
    Wlin1 = np.asarray(inputs["Wlin1"], np.float32)

    elem = np.argmax(y, axis=1)
    lay = _layout(elem)
    NSLOT = lay["NSLOT"]
    rt = _get_rt(lay["eb"])
    dev = _get_dev_consts(inputs, rt)

    # one fused scatter: xt4d[core, i, slot, c] = x[node, c, i]; pad slots
    # stay zero from allocation (node slots are overwritten every call)
    xt4d = lay.setdefault(
        "xt4d", np.zeros((NCORES, I, NSLOT, C), np.float16))
    xt4d[lay["core_of"], :, lay["slot_of"]] = x.swapaxes(1, 2)
    xt16 = xt4d.reshape(NCORES * I, lay["FT"])

    args = [xt16 if nm == "XT" else dev[nm] for nm in rt["in_names"]]
    out = rt["sharded"](*args, *rt["zeros"])
    f_np = np.asarray(out[0])                      # [NCORES*4, FT] f16
    fall = f_np.reshape(NCORES, 4, NSLOT, C)

    # fg[n, d, c] = f value for node n, irrep-component d, channel c
    fg = fall[lay["core_of"], :, lay["slot_of"], :].astype(np.float32)

    inv = np.float32(1.0 / np.sqrt(C))
    out_np = np.empty((N, C * 4), np.float32)
    np.matmul(fg[:, 0, :], Wlin0, out=out_np[:, :C])
    z = (fg[:, 1:, :].reshape(N * 3, C) @ Wlin1).reshape(N, 3, C)
    out_np[:, C:] = z.transpose(0, 2, 1).reshape(N, 3 * C)
    out_np *= inv
    out_np += sc
    return out_np


def kernel(**inputs):
    res = _run(inputs)
    import os
    nrep = int(os.environ.get("KERNEL_TIME_RUNS", "0"))
    if nrep:
        import time
        times = []
        for _ in range(nrep):
            t0 = time.perf_counter()
            _run(inputs)
            times.append(time.perf_counter() - t0)
        globals()["LAST_TIMES"] = times
    return res


# revision 25
# speedup vs baseline: 1.0176x; 1.0018x over previous
"""Trainium2 Bass kernel for nn_EquivariantProductBasisBlock.

Math: per (n,c) with x = node_feats[n,c,:] in R^9, one-hot node_attrs:
  f[n,c,dt] = sum_k w3[n,k,c] * <U3sym[dt,:,k], mono3(x)>
            + sum_k w2[n,k,c] * <U2sym[dt,:,k], mono2(x)>
            + sum_k w1[n,k,c] * <U1[dt,:,k], x>
  out = concat_dt(f @ Wlin) / sqrt(C) + sc

The device builds the symmetric monomial basis itself from xT [9, F]
(the axon tunnel runs ~40 MB/s, so shipping host-staged monomials is
the bottleneck; shipping only xT cuts H2D ~25x). Per 256-col block
(2 node-slots x 128 channels, c-fastest):
  m2[45,F]   = (A2^T xT) * (B2^T xT)            (PE gathers + DVE mul)
  m3[165,F]  = (A3^T xT) * (P23^T m2)           (split 128+37 rows)
  G[124,F]   = CFa^T m3a + CFb3^T m3b + CF2^T m2  (PE, 3-way accum)
  T1         = G * WE32[elem]                    (DVE, c-broadcast AP)
  T1u        = (S1u^T xT) * WE1[elem]            (PE + DVE)
  f[4,F]     = R1^T T1 + R2^T T1u                (PE k-reduction)
Nodes are dealt to cores round-robin per element class so the
block->element map is identical on all 8 cores (SPMD-uniform).
Dispatch is a cached jax.jit(shard_map(bass_exec)) — U/W-derived
constants and zero output buffers live on device across calls; only
xT (fp16) is shipped per call and only f (fp16) is fetched back.
Host: scatter/gather permutation, final equivariant Linear + sc.
"""
import sys
import numpy as np

sys.path.insert(0, "/opt/trn_rl_repo")

N, C, I, E = 2048, 128, 9, 10
K3, K2, K1 = 23, 8, 3
NCORES = 8
FB = 256                  # free cols per block
SLOTS_PER_BLK = FB // C   # 2 node-slots per block

TRI3 = [(a, b, c) for a in range(I) for b in range(a, I) for c in range(b, I)]
TRI2 = [(a, b) for a in range(I) for b in range(a, I)]
M2IDX = {ab: r for r, ab in enumerate(TRI2)}
NM3, NM2 = len(TRI3), len(TRI2)           # 165, 45
NC3, NC2, NC1 = 4 * K3, 4 * K2, 4 * K1    # 92, 32, 12
NCOL = NC3 + NC2                          # 124
MAR = 128                                 # m3 rows in the A split
MBR3 = NM3 - MAR                          # 37
DT_LIST = [(0, 0), (1, 0), (1, 1), (1, 2)]

_RT = {}            # eb tuple -> runtime dict (nc, jitted dispatch, zeros)
_DEV_CONSTS = {}    # (consts md5) -> name -> sharded device array
_LAYOUT = {}        # (elem md5) -> layout dict


def _build_consts(inputs):
    """Coefficient / weight matrices derived from the U/W input tensors."""
    U3s = [np.asarray(inputs["U3_0"]), np.asarray(inputs["U3_1"])]
    U2s = [np.asarray(inputs["U2_0"]), np.asarray(inputs["U2_1"])]
    U1s = [np.asarray(inputs["U1_0"]), np.asarray(inputs["U1_1"])]
    W3s = [np.asarray(inputs["W3_0"]), np.asarray(inputs["W3_1"])]
    W2s = [np.asarray(inputs["W2_0"]), np.asarray(inputs["W2_1"])]
    W1s = [np.asarray(inputs["W1_0"]), np.asarray(inputs["W1_1"])]

    # symmetrized U3/U2 -> CF [mono-row, (dt,k) col]
    CF3 = np.zeros((NM3, NCOL), np.float64)
    CF2 = np.zeros((NM2, NCOL), np.float64)
    tri3_idx = {m: r for r, m in enumerate(TRI3)}
    for di, (s, d) in enumerate(DT_LIST):
        u3 = np.zeros((NM3, K3), np.float64)
        u2 = np.zeros((NM2, K2), np.float64)
        U3 = np.asarray(U3s[s], np.float64)
        U2 = np.asarray(U2s[s], np.float64)
        for p in range(I):
            for q in range(I):
                u2[M2IDX[tuple(sorted((p, q)))]] += U2[d, p, q, :]
                for i in range(I):
                    u3[tri3_idx[tuple(sorted((p, q, i)))]] += U3[d, p, q, i, :]
        CF3[:, di * K3:(di + 1) * K3] = u3
        CF2[:, NC3 + di * K2:NC3 + (di + 1) * K2] = u2

    CFall = np.concatenate([CF3, CF2], axis=0)   # [210, 124]
    S1u = np.zeros((I, NC1), np.float32)         # U1 fold: U1X = S1u.T @ xT
    for di, (s, d) in enumerate(DT_LIST):
        S1u[:, di * K1:(di + 1) * K1] = U1s[s][d, :, :]

    R1 = np.zeros((NCOL, 4), np.float16)
    R2 = np.zeros((NC1, 4), np.float16)
    WE32 = np.zeros((NCOL, E, C), np.float32)
    WE1 = np.zeros((NC1, E, C), np.float32)
    for di, (s, d) in enumerate(DT_LIST):
        R1[di * K3:(di + 1) * K3, di] = 1.0
        R1[NC3 + di * K2:NC3 + (di + 1) * K2, di] = 1.0
        R2[di * K1:(di + 1) * K1, di] = 1.0
        WE32[di * K3:(di + 1) * K3] = W3s[s].transpose(1, 0, 2)
        WE32[NC3 + di * K2:NC3 + (di + 1) * K2] = W2s[s].transpose(1, 0, 2)
        WE1[di * K1:(di + 1) * K1] = W1s[s].transpose(1, 0, 2)

    # partition-gather matrices for on-device monomial construction
    A2 = np.zeros((I, NM2), np.float16)
    B2 = np.zeros((I, NM2), np.float16)
    for r, (a, b) in enumerate(TRI2):
        A2[a, r] = 1.0
        B2[b, r] = 1.0
    A3 = np.zeros((I, NM3), np.float16)
    P23 = np.zeros((NM2, NM3), np.float16)
    for r, (a, b, c) in enumerate(TRI3):
        A3[a, r] = 1.0
        P23[M2IDX[(b, c)], r] = 1.0

    return {
        "A2c": A2, "B2c": B2,
        "A3a": A3[:, :MAR].copy(), "A3b": A3[:, MAR:].copy(),
        "P23a": P23[:, :MAR].copy(), "P23b": P23[:, MAR:].copy(),
        "S1uc": S1u.astype(np.float16),
        "CFa": CFall[:MAR].astype(np.float16),
        "CFb3": CFall[MAR:NM3].astype(np.float16),
        "CF2": CFall[NM3:].astype(np.float16),
        "R1": R1, "R2": R2,
        "WE32": WE32.reshape(NCOL, E * C).astype(np.float16),
        "WE1": WE1.reshape(NC1, E * C).astype(np.float16),
    }


def _build_nc(eb):
    """Bass program; eb = element id per block (same on all cores)."""
    from concourse import bass, bacc, tile, mybir

    f32 = mybir.dt.float32
    f16 = mybir.dt.float16
    NBLK = len(eb)
    FT = NBLK * FB

    nc = bacc.Bacc(None, target_bir_lowering=False, debug=False)
    xt_d = nc.declare_dram_parameter("XT", [I, FT], f16, isOutput=False)
    cshapes = {
        "A2c": ([I, NM2], f16), "B2c": ([I, NM2], f16),
        "A3a": ([I, MAR], f16), "A3b": ([I, MBR3], f16),
        "P23a": ([NM2, MAR], f16), "P23b": ([NM2, MBR3], f16),
        "S1uc": ([I, NC1], f16),
        "CFa": ([MAR, NCOL], f16), "CFb3": ([MBR3, NCOL], f16),
        "CF2": ([NM2, NCOL], f16),
        "R1": ([NCOL, 4], f16), "R2": ([NC1, 4], f16),
        "WE32": ([NCOL, E * C], f16), "WE1": ([NC1, E * C], f16),
    }
    cd = {k: nc.declare_dram_parameter(k, shp, dt, isOutput=False)
          for k, (shp, dt) in cshapes.items()}
    f_d = nc.declare_dram_parameter("f", [4, FT], f16, isOutput=True)

    with tile.TileContext(nc) as tc:
        with (
            tc.tile_pool(name="const", bufs=1) as cpool,
            tc.tile_pool(name="xin", bufs=2) as xpool,
            tc.tile_pool(name="work", bufs=3) as wpool,
            tc.tile_pool(name="stage", bufs=2) as spool,
            tc.tile_pool(name="psg", bufs=4, space=bass.MemorySpace.PSUM) as gp,
            tc.tile_pool(name="psG", bufs=2, space=bass.MemorySpace.PSUM) as pG,
            tc.tile_pool(name="psF", bufs=2, space=bass.MemorySpace.PSUM) as pF,
        ):
            ct = {}
            for k, (shp, dt) in cshapes.items():
                ct[k] = cpool.tile(shp, dt, tag=k, name=k)
                nc.sync.dma_start(out=ct[k][:], in_=cd[k][:])

            XCHUNK = 4096 // FB  # blocks per x DMA chunk
            for b in range(NBLK):
                if b % XCHUNK == 0:
                    w = min(XCHUNK * FB, FT - b * FB)
                    xch = xpool.tile([I, XCHUNK * FB], f16, tag="xch")
                    nc.sync.dma_start(out=xch[:, :w],
                                      in_=xt_d[:, b * FB:b * FB + w])
                    fstage = spool.tile([4, XCHUNK * FB], f16, tag="fst")
                o = (b % XCHUNK) * FB
                xt = xch[:, o:o + FB]
                e = eb[b]

                # m2 = (A2^T xT) * (B2^T xT)
                ga2 = gp.tile([NM2, FB], f32, tag="gat")
                nc.tensor.matmul(ga2[:], ct["A2c"][:], xt, start=True, stop=True)
                ca2 = wpool.tile([NM2, FB], f16, tag="ca2")
                nc.scalar.copy(ca2[:], ga2[:])
                gb2 = gp.tile([NM2, FB], f32, tag="gat")
                nc.tensor.matmul(gb2[:], ct["B2c"][:], xt, start=True, stop=True)
                m2 = wpool.tile([NM2, FB], f16, tag="m2")
                nc.vector.tensor_mul(m2[:], ca2[:], gb2[:])

                # m3 rows 0..127
                ga3a = gp.tile([MAR, FB], f32, tag="gat")
                nc.tensor.matmul(ga3a[:], ct["A3a"][:], xt, start=True, stop=True)
                ca3a = wpool.tile([MAR, FB], f16, tag="ca3a")
                nc.scalar.copy(ca3a[:], ga3a[:])
                gm2a = gp.tile([MAR, FB], f32, tag="gat")
                nc.tensor.matmul(gm2a[:], ct["P23a"][:], m2[:], start=True, stop=True)
                ma = wpool.tile([MAR, FB], f16, tag="ma")
                nc.vector.tensor_mul(ma[:], ca3a[:], gm2a[:])

                # m3 rows 128..164
                ga3b = gp.tile([MBR3, FB], f32, tag="gat")
                nc.tensor.matmul(ga3b[:], ct["A3b"][:], xt, start=True, stop=True)
                ca3b = wpool.tile([MBR3, FB], f16, tag="ca3b")
                nc.scalar.copy(ca3b[:], ga3b[:])
                gm2b = gp.tile([MBR3, FB], f32, tag="gat")
                nc.tensor.matmul(gm2b[:], ct["P23b"][:], m2[:], start=True, stop=True)
                mb = wpool.tile([MBR3, FB], f16, tag="mb")
                nc.vector.tensor_mul(mb[:], ca3b[:], gm2b[:])

                # T1u = (S1u^T xT) * WE1[elem]
                ux = gp.tile([NC1, FB], f32, tag="gat")
                nc.tensor.matmul(ux[:], ct["S1uc"][:], xt, start=True, stop=True)
                we1 = ct["WE1"][:, e * C:(e + 1) * C]
                we1b = we1.unsqueeze(1).broadcast_to([NC1, SLOTS_PER_BLK, C])
                t1u = wpool.tile([NC1, SLOTS_PER_BLK, C], f16, tag="t1u")
                nc.vector.tensor_mul(
                    t1u[:],
                    ux[:].rearrange("p (n c) -> p n c", n=SLOTS_PER_BLK), we1b)

                # G = CFa^T m3a + CFb3^T m3b + CF2^T m2
                g = pG.tile([NCOL, FB], f32, tag="g")
                nc.tensor.matmul(g[:], ct["CFa"][:], ma[:], start=True, stop=False)
                nc.tensor.matmul(g[:], ct["CFb3"][:], mb[:], start=False, stop=False)
                nc.tensor.matmul(g[:], ct["CF2"][:], m2[:], start=False, stop=True)

                we = ct["WE32"][:, e * C:(e + 1) * C]
                web = we.unsqueeze(1).broadcast_to([NCOL, SLOTS_PER_BLK, C])
                t1 = wpool.tile([NCOL, SLOTS_PER_BLK, C], f16, tag="t1")
                nc.vector.tensor_mul(
                    t1[:],
                    g[:].rearrange("p (n c) -> p n c", n=SLOTS_PER_BLK), web)

                f_ps = pF.tile([4, FB], f32, tag="f")
                nc.tensor.matmul(f_ps[:], ct["R1"][:],
                                 t1[:].rearrange("p n c -> p (n c)"),
                                 start=True, stop=False)
                nc.tensor.matmul(f_ps[:], ct["R2"][:],
                                 t1u[:].rearrange("p n c -> p (n c)"),
                                 start=False, stop=True)
                nc.scalar.copy(fstage[:, o:o + FB], f_ps[:])
                if b % XCHUNK == XCHUNK - 1 or b == NBLK - 1:
                    lo = (b // XCHUNK) * XCHUNK
                    w = (b - lo + 1) * FB
                    nc.sync.dma_start(out=f_d[:, lo * FB:lo * FB + w],
                                      in_=fstage[:, :w])

    nc.compile()
    return nc


def _layout(elem):
    """Node -> (core, slot) dealing; identical block->element map per core."""
    key = elem.tobytes()
    if key in _LAYOUT:
        return _LAYOUT[key]
    count = np.bincount(elem, minlength=E)
    spe = [int(np.ceil(c / NCORES)) if c else 0 for c in count]
    blocks_e = [int(np.ceil(s / SLOTS_PER_BLK)) for s in spe]
    eb = []
    base_slot = []
    for e in range(E):
        base_slot.append(len(eb) * SLOTS_PER_BLK)
        eb.extend([e] * blocks_e[e])
    NBLK = len(eb)
    NSLOT = NBLK * SLOTS_PER_BLK
    order = np.argsort(elem, kind="stable")
    core_of = np.empty(N, np.int64)
    slot_of = np.empty(N, np.int64)
    pos = 0
    for e in range(E):
        idx = order[pos:pos + count[e]]
        pos += count[e]
        j = np.arange(count[e])
        core_of[idx] = j % NCORES
        slot_of[idx] = base_slot[e] + j // NCORES
    # gather index: gidx[core, slot] = node id, or N for padding
    gidx = np.full((NCORES, NSLOT), N, np.int64)
    gidx[core_of, slot_of] = np.arange(N)
    lay = {"eb": tuple(eb), "NBLK": NBLK, "NSLOT": NSLOT, "FT": NBLK * FB,
           "core_of": core_of, "slot_of": slot_of, "gidx": gidx}
    _LAYOUT[key] = lay
    return lay


def _get_rt(eb):
    """Compile the Bass program and build the cached jitted dispatch."""
    key = tuple(eb)
    if key in _RT:
        return _RT[key]
    import jax
    from jax.sharding import Mesh, PartitionSpec, NamedSharding
    from jax.experimental.shard_map import shard_map
    from concourse import mybir
    from concourse.bass2jax import (_bass_exec_p, install_neuronx_cc_hook,
                                    partition_id_tensor)

    install_neuronx_cc_hook()
    nc = _build_nc(list(key))

    partition_name = nc.partition_id_tensor.name if nc.partition_id_tensor else None
    in_names, out_names, out_avals, zero_shapes = [], [], [], []
    for alloc in nc.m.functions[0].allocations:
        if not isinstance(alloc, mybir.MemoryLocationSet):
            continue
        name = alloc.memorylocations[0].name
        if alloc.kind == "ExternalInput":
            if name != partition_name:
                in_names.append(name)
        elif alloc.kind == "ExternalOutput":
            out_names.append(name)
            shape = tuple(alloc.tensor_shape)
            dtype = mybir.dt.np(alloc.dtype)
            out_avals.append(jax.core.ShapedArray(shape, dtype))
            zero_shapes.append((shape, dtype))
    n_params = len(in_names)
    in_names_full = in_names + out_names + (
        [partition_name] if partition_name else [])

    def _body(*args):
        operands = list(args)
        if partition_name is not None:
            operands.append(partition_id_tensor())
        outs = _bass_exec_p.bind(
            *operands, out_avals=tuple(out_avals),
            in_names=tuple(in_names_full), out_names=tuple(out_names),
            lowering_input_output_aliases=(), sim_require_finite=True,
            sim_require_nnan=True, nc=nc)
        return tuple(outs)

    devices = jax.devices()[:NCORES]
    mesh = Mesh(np.asarray(devices), ("core",))
    nin = n_params + len(out_names)
    sh = NamedSharding(mesh, PartitionSpec("core"))

    def _make_jit():
        return jax.jit(
            shard_map(_body, mesh=mesh,
                      in_specs=(PartitionSpec("core"),) * nin,
                      out_specs=(PartitionSpec("core"),) * len(out_names),
                      check_rep=False),
            keep_unused=True)

    # abstract avals for AOT lowering (global shapes, sharded on axis 0)
    name2shape = {}
    for alloc in nc.m.functions[0].allocations:
        if isinstance(alloc, mybir.MemoryLocationSet) and alloc.tensor_shape:
            from concourse import mybir as _mb
            name2shape[alloc.memorylocations[0].name] = (
                tuple(alloc.tensor_shape), _mb.dt.np(alloc.dtype))
    structs = []
    for nm in in_names + out_names:
        shp, dt = name2shape[nm]
        structs.append(jax.ShapeDtypeStruct(
            (NCORES * shp[0],) + tuple(shp[1:]), dt, sharding=sh))
    try:
        from concourse.bass2jax import fast_dispatch_compile
        sharded = fast_dispatch_compile(
            lambda: _make_jit().lower(*structs).compile())
    except Exception:
        sharded = _make_jit()
    # zero output operands live on device; not donated, so reusable forever
    zeros = [jax.device_put(
        np.zeros((NCORES * s[0], *s[1:]), dt), sh) for s, dt in zero_shapes]
    rt = {"nc": nc, "sharded": sharded, "in_names": in_names,
          "out_names": out_names, "zeros": zeros, "sh": sh, "mesh": mesh}
    _RT[key] = rt
    return rt


def _get_dev_consts(inputs, rt):
    import jax
    import hashlib
    h = hashlib.md5()
    for k in ("U3_0", "U2_0", "U1_0", "W3_0", "W2_0", "W1_0",
              "U3_1", "U2_1", "U1_1", "W3_1", "W2_1", "W1_1"):
        h.update(np.ascontiguousarray(np.asarray(inputs[k])).tobytes())
    key = h.hexdigest()
    if key in _DEV_CONSTS:
        return _DEV_CONSTS[key]
    consts = _build_consts(inputs)
    dev = {k: jax.device_put(np.tile(v, (NCORES,) + (1,) * (v.ndim - 1)),
                             rt["sh"]) for k, v in consts.items()}
    for z in dev.values():
        z.block_until_ready()
    _DEV_CONSTS[key] = dev
    return dev


def _run(inputs):
    x = np.asarray(inputs["node_feats"], np.float32)
    sc = np.asarray(inputs["sc"], np.float32)
    y = np.asarray(inputs["node_attrs"], np.float32)
    Wlin0 = np.asarray(inputs["Wlin0"], np.float32)
    Wlin1 = np.asarray(inputs["Wlin1"], np.float32)

    elem = np.argmax(y, axis=1)
    lay = _layout(elem)
    NSLOT = lay["NSLOT"]
    rt = _get_rt(lay["eb"])
    dev = _get_dev_consts(inputs, rt)

    # one fused scatter: xt4d[core, i, slot, c] = x[node, c, i]; pad slots
    # stay zero from allocation (node slots are overwritten every call)
    xt4d = lay.setdefault(
        "xt4d", np.zeros((NCORES, I, NSLOT, C), np.float16))
    xt4d[lay["core_of"], :, lay["slot_of"]] = x.swapaxes(1, 2)
    xt16 = xt4d.reshape(NCORES * I, lay["FT"])

    args = [xt16 if nm == "XT" else dev[nm] for nm in rt["in_names"]]
    out = rt["sharded"](*args, *rt["zeros"])
    f_np = np.asarray(out[0])                      # [NCORES*4, FT] f16
    fall = f_np.reshape(NCORES, 4, NSLOT, C)

    # fg[n, d, c] = f value for node n, irrep-component d, channel c
    fg = fall[lay["core_of"], :, lay["slot_of"], :].astype(np.float32)

    inv = np.float32(1.0 / np.sqrt(C))
    out_np = np.empty((N, C * 4), np.float32)
    np.matmul(fg[:, 0, :], Wlin0, out=out_np[:, :C])
    z = (fg[:, 1:, :].reshape(N * 3, C) @ Wlin1).reshape(N, 3, C)
    out_np[:, C:] = z.transpose(0, 2, 1).reshape(N, 3 * C)
    out_np *= inv
    out_np += sc
    return out_np


def kernel(**inputs):
    res = _run(inputs)
    import os
    nrep = int(os.environ.get("KERNEL_TIME_RUNS", "0"))
    if nrep:
        import time
        times = []
        for _ in range(nrep):
            t0 = time.perf_counter()
            _run(inputs)
            times.append(time.perf_counter() - t0)
        globals()["LAST_TIMES"] = times
    return res


# revision 27
# speedup vs baseline: 1.0813x; 1.0626x over previous
"""Trainium2 Bass kernel for nn_EquivariantProductBasisBlock.

Math: per (n,c) with x = node_feats[n,c,:] in R^9, one-hot node_attrs:
  f[n,c,dt] = sum_k w3[n,k,c] * <U3sym[dt,:,k], mono3(x)>
            + sum_k w2[n,k,c] * <U2sym[dt,:,k], mono2(x)>
            + sum_k w1[n,k,c] * <U1[dt,:,k], x>
  out = concat_dt(f @ Wlin) / sqrt(C) + sc

The device builds the symmetric monomial basis itself from xT [9, F]
(the axon tunnel runs ~40 MB/s, so shipping host-staged monomials is
the bottleneck; shipping only xT cuts H2D ~25x). Per 256-col block
(2 node-slots x 128 channels, c-fastest):
  m2[45,F]   = (A2^T xT) * (B2^T xT)            (PE gathers + DVE mul)
  m3[165,F]  = (A3^T xT) * (P23^T m2)           (split 128+37 rows)
  G[124,F]   = CFa^T m3a + CFb3^T m3b + CF2^T m2  (PE, 3-way accum)
  T1         = G * WE32[elem]                    (DVE, c-broadcast AP)
  T1u        = (S1u^T xT) * WE1[elem]            (PE + DVE)
  f[4,F]     = R1^T T1 + R2^T T1u                (PE k-reduction)
Nodes are dealt to cores round-robin per element class so the
block->element map is identical on all 8 cores (SPMD-uniform).
Dispatch is a cached jax.jit(shard_map(bass_exec)) — U/W-derived
constants and zero output buffers live on device across calls; only
xT (fp16) is shipped per call and only f (fp16) is fetched back.
Host: scatter/gather permutation, final equivariant Linear + sc.
"""
import sys
import numpy as np

sys.path.insert(0, "/opt/trn_rl_repo")

N, C, I, E = 2048, 128, 9, 10
K3, K2, K1 = 23, 8, 3
NCORES = 8
FB = 256                  # free cols per block
SLOTS_PER_BLK = FB // C   # 2 node-slots per block

TRI3 = [(a, b, c) for a in range(I) for b in range(a, I) for c in range(b, I)]
TRI2 = [(a, b) for a in range(I) for b in range(a, I)]
M2IDX = {ab: r for r, ab in enumerate(TRI2)}
NM3, NM2 = len(TRI3), len(TRI2)           # 165, 45
NC3, NC2, NC1 = 4 * K3, 4 * K2, 4 * K1    # 92, 32, 12
NCOL = NC3 + NC2                          # 124
MAR = 128                                 # m3 rows in the A split
MBR3 = NM3 - MAR                          # 37
DT_LIST = [(0, 0), (1, 0), (1, 1), (1, 2)]

_RT = {}            # eb tuple -> runtime dict (nc, jitted dispatch, zeros)
_DEV_CONSTS = {}    # (consts md5) -> name -> sharded device array
_LAYOUT = {}        # (elem md5) -> layout dict


def _build_consts(inputs):
    """Coefficient / weight matrices derived from the U/W input tensors."""
    U3s = [np.asarray(inputs["U3_0"]), np.asarray(inputs["U3_1"])]
    U2s = [np.asarray(inputs["U2_0"]), np.asarray(inputs["U2_1"])]
    U1s = [np.asarray(inputs["U1_0"]), np.asarray(inputs["U1_1"])]
    W3s = [np.asarray(inputs["W3_0"]), np.asarray(inputs["W3_1"])]
    W2s = [np.asarray(inputs["W2_0"]), np.asarray(inputs["W2_1"])]
    W1s = [np.asarray(inputs["W1_0"]), np.asarray(inputs["W1_1"])]

    # symmetrized U3/U2 -> CF [mono-row, (dt,k) col]
    CF3 = np.zeros((NM3, NCOL), np.float64)
    CF2 = np.zeros((NM2, NCOL), np.float64)
    tri3_idx = {m: r for r, m in enumerate(TRI3)}
    for di, (s, d) in enumerate(DT_LIST):
        u3 = np.zeros((NM3, K3), np.float64)
        u2 = np.zeros((NM2, K2), np.float64)
        U3 = np.asarray(U3s[s], np.float64)
        U2 = np.asarray(U2s[s], np.float64)
        for p in range(I):
            for q in range(I):
                u2[M2IDX[tuple(sorted((p, q)))]] += U2[d, p, q, :]
                for i in range(I):
                    u3[tri3_idx[tuple(sorted((p, q, i)))]] += U3[d, p, q, i, :]
        CF3[:, di * K3:(di + 1) * K3] = u3
        CF2[:, NC3 + di * K2:NC3 + (di + 1) * K2] = u2

    CFall = np.concatenate([CF3, CF2], axis=0)   # [210, 124]
    S1u = np.zeros((I, NC1), np.float32)         # U1 fold: U1X = S1u.T @ xT
    for di, (s, d) in enumerate(DT_LIST):
        S1u[:, di * K1:(di + 1) * K1] = U1s[s][d, :, :]

    R1 = np.zeros((NCOL, 4), np.float16)
    R2 = np.zeros((NC1, 4), np.float16)
    WE32 = np.zeros((NCOL, E, C), np.float32)
    WE1 = np.zeros((NC1, E, C), np.float32)
    for di, (s, d) in enumerate(DT_LIST):
        R1[di * K3:(di + 1) * K3, di] = 1.0
        R1[NC3 + di * K2:NC3 + (di + 1) * K2, di] = 1.0
        R2[di * K1:(di + 1) * K1, di] = 1.0
        WE32[di * K3:(di + 1) * K3] = W3s[s].transpose(1, 0, 2)
        WE32[NC3 + di * K2:NC3 + (di + 1) * K2] = W2s[s].transpose(1, 0, 2)
        WE1[di * K1:(di + 1) * K1] = W1s[s].transpose(1, 0, 2)

    # partition-gather matrices for on-device monomial construction
    A2 = np.zeros((I, NM2), np.float16)
    B2 = np.zeros((I, NM2), np.float16)
    for r, (a, b) in enumerate(TRI2):
        A2[a, r] = 1.0
        B2[b, r] = 1.0
    A3 = np.zeros((I, NM3), np.float16)
    P23 = np.zeros((NM2, NM3), np.float16)
    for r, (a, b, c) in enumerate(TRI3):
        A3[a, r] = 1.0
        P23[M2IDX[(b, c)], r] = 1.0

    return {
        "A2c": A2, "B2c": B2,
        "A3a": A3[:, :MAR].copy(), "A3b": A3[:, MAR:].copy(),
        "P23a": P23[:, :MAR].copy(), "P23b": P23[:, MAR:].copy(),
        "S1uc": S1u.astype(np.float16),
        "CFa": CFall[:MAR].astype(np.float16),
        "CFb3": CFall[MAR:NM3].astype(np.float16),
        "CF2": CFall[NM3:].astype(np.float16),
        "R1": R1, "R2": R2,
        "WE32": WE32.reshape(NCOL, E * C).astype(np.float16),
        "WE1": WE1.reshape(NC1, E * C).astype(np.float16),
    }


def _build_nc(eb):
    """Bass program; eb = element id per block (same on all cores)."""
    from concourse import bass, bacc, tile, mybir

    f32 = mybir.dt.float32
    f16 = mybir.dt.float16
    NBLK = len(eb)
    FT = NBLK * FB

    nc = bacc.Bacc(None, target_bir_lowering=False, debug=False)
    xt_d = nc.declare_dram_parameter("XT", [I, FT], f16, isOutput=False)
    cshapes = {
        "A2c": ([I, NM2], f16), "B2c": ([I, NM2], f16),
        "A3a": ([I, MAR], f16), "A3b": ([I, MBR3], f16),
        "P23a": ([NM2, MAR], f16), "P23b": ([NM2, MBR3], f16),
        "S1uc": ([I, NC1], f16),
        "CFa": ([MAR, NCOL], f16), "CFb3": ([MBR3, NCOL], f16),
        "CF2": ([NM2, NCOL], f16),
        "R1": ([NCOL, 4], f16), "R2": ([NC1, 4], f16),
        "WE32": ([NCOL, E * C], f16), "WE1": ([NC1, E * C], f16),
    }
    cd = {k: nc.declare_dram_parameter(k, shp, dt, isOutput=False)
          for k, (shp, dt) in cshapes.items()}
    f_d = nc.declare_dram_parameter("f", [4, FT], f16, isOutput=True)

    with tile.TileContext(nc) as tc:
        with (
            tc.tile_pool(name="const", bufs=1) as cpool,
            tc.tile_pool(name="xin", bufs=2) as xpool,
            tc.tile_pool(name="work", bufs=3) as wpool,
            tc.tile_pool(name="stage", bufs=2) as spool,
            tc.tile_pool(name="psg", bufs=4, space=bass.MemorySpace.PSUM) as gp,
            tc.tile_pool(name="psG", bufs=2, space=bass.MemorySpace.PSUM) as pG,
            tc.tile_pool(name="psF", bufs=2, space=bass.MemorySpace.PSUM) as pF,
        ):
            ct = {}
            for k, (shp, dt) in cshapes.items():
                ct[k] = cpool.tile(shp, dt, tag=k, name=k)
                nc.sync.dma_start(out=ct[k][:], in_=cd[k][:])

            XCHUNK = 4096 // FB  # blocks per x DMA chunk
            for b in range(NBLK):
                if b % XCHUNK == 0:
                    w = min(XCHUNK * FB, FT - b * FB)
                    xch = xpool.tile([I, XCHUNK * FB], f16, tag="xch")
                    nc.sync.dma_start(out=xch[:, :w],
                                      in_=xt_d[:, b * FB:b * FB + w])
                    fstage = spool.tile([4, XCHUNK * FB], f16, tag="fst")
                o = (b % XCHUNK) * FB
                xt = xch[:, o:o + FB]
                e = eb[b]

                # m2 = (A2^T xT) * (B2^T xT)
                ga2 = gp.tile([NM2, FB], f32, tag="gat")
                nc.tensor.matmul(ga2[:], ct["A2c"][:], xt, start=True, stop=True)
                ca2 = wpool.tile([NM2, FB], f16, tag="ca2")
                nc.scalar.copy(ca2[:], ga2[:])
                gb2 = gp.tile([NM2, FB], f32, tag="gat")
                nc.tensor.matmul(gb2[:], ct["B2c"][:], xt, start=True, stop=True)
                m2 = wpool.tile([NM2, FB], f16, tag="m2")
                nc.vector.tensor_mul(m2[:], ca2[:], gb2[:])

                # m3 rows 0..127
                ga3a = gp.tile([MAR, FB], f32, tag="gat")
                nc.tensor.matmul(ga3a[:], ct["A3a"][:], xt, start=True, stop=True)
                ca3a = wpool.tile([MAR, FB], f16, tag="ca3a")
                nc.scalar.copy(ca3a[:], ga3a[:])
                gm2a = gp.tile([MAR, FB], f32, tag="gat")
                nc.tensor.matmul(gm2a[:], ct["P23a"][:], m2[:], start=True, stop=True)
                ma = wpool.tile([MAR, FB], f16, tag="ma")
                nc.vector.tensor_mul(ma[:], ca3a[:], gm2a[:])

                # m3 rows 128..164
                ga3b = gp.tile([MBR3, FB], f32, tag="gat")
                nc.tensor.matmul(ga3b[:], ct["A3b"][:], xt, start=True, stop=True)
                ca3b = wpool.tile([MBR3, FB], f16, tag="ca3b")
                nc.scalar.copy(ca3b[:], ga3b[:])
                gm2b = gp.tile([MBR3, FB], f32, tag="gat")
                nc.tensor.matmul(gm2b[:], ct["P23b"][:], m2[:], start=True, stop=True)
                mb = wpool.tile([MBR3, FB], f16, tag="mb")
                nc.vector.tensor_mul(mb[:], ca3b[:], gm2b[:])

                # T1u = (S1u^T xT) * WE1[elem]
                ux = gp.tile([NC1, FB], f32, tag="gat")
                nc.tensor.matmul(ux[:], ct["S1uc"][:], xt, start=True, stop=True)
                we1 = ct["WE1"][:, e * C:(e + 1) * C]
                we1b = we1.unsqueeze(1).broadcast_to([NC1, SLOTS_PER_BLK, C])
                t1u = wpool.tile([NC1, SLOTS_PER_BLK, C], f16, tag="t1u")
                nc.vector.tensor_mul(
                    t1u[:],
                    ux[:].rearrange("p (n c) -> p n c", n=SLOTS_PER_BLK), we1b)

                # G = CFa^T m3a + CFb3^T m3b + CF2^T m2
                g = pG.tile([NCOL, FB], f32, tag="g")
                nc.tensor.matmul(g[:], ct["CFa"][:], ma[:], start=True, stop=False)
                nc.tensor.matmul(g[:], ct["CFb3"][:], mb[:], start=False, stop=False)
                nc.tensor.matmul(g[:], ct["CF2"][:], m2[:], start=False, stop=True)

                we = ct["WE32"][:, e * C:(e + 1) * C]
                web = we.unsqueeze(1).broadcast_to([NCOL, SLOTS_PER_BLK, C])
                t1 = wpool.tile([NCOL, SLOTS_PER_BLK, C], f16, tag="t1")
                nc.vector.tensor_mul(
                    t1[:],
                    g[:].rearrange("p (n c) -> p n c", n=SLOTS_PER_BLK), web)

                f_ps = pF.tile([4, FB], f32, tag="f")
                nc.tensor.matmul(f_ps[:], ct["R1"][:],
                                 t1[:].rearrange("p n c -> p (n c)"),
                                 start=True, stop=False)
                nc.tensor.matmul(f_ps[:], ct["R2"][:],
                                 t1u[:].rearrange("p n c -> p (n c)"),
                                 start=False, stop=True)
                nc.scalar.copy(fstage[:, o:o + FB], f_ps[:])
                if b % XCHUNK == XCHUNK - 1 or b == NBLK - 1:
                    lo = (b // XCHUNK) * XCHUNK
                    w = (b - lo + 1) * FB
                    nc.sync.dma_start(out=f_d[:, lo * FB:lo * FB + w],
                                      in_=fstage[:, :w])

    nc.compile()
    return nc


def _layout(elem):
    """Node -> (core, slot) dealing; identical block->element map per core."""
    key = elem.tobytes()
    if key in _LAYOUT:
        return _LAYOUT[key]
    count = np.bincount(elem, minlength=E)
    spe = [int(np.ceil(c / NCORES)) if c else 0 for c in count]
    blocks_e = [int(np.ceil(s / SLOTS_PER_BLK)) for s in spe]
    eb = []
    base_slot = []
    for e in range(E):
        base_slot.append(len(eb) * SLOTS_PER_BLK)
        eb.extend([e] * blocks_e[e])
    NBLK = len(eb)
    NSLOT = NBLK * SLOTS_PER_BLK
    order = np.argsort(elem, kind="stable")
    core_of = np.empty(N, np.int64)
    slot_of = np.empty(N, np.int64)
    pos = 0
    for e in range(E):
        idx = order[pos:pos + count[e]]
        pos += count[e]
        j = np.arange(count[e])
        core_of[idx] = j % NCORES
        slot_of[idx] = base_slot[e] + j // NCORES
    # gather index: gidx[core, slot] = node id, or N for padding
    gidx = np.full((NCORES, NSLOT), N, np.int64)
    gidx[core_of, slot_of] = np.arange(N)
    nodes_by_core = [np.nonzero(core_of == c)[0] for c in range(NCORES)]
    slots_by_core = [slot_of[nodes_by_core[c]] for c in range(NCORES)]
    lay = {"eb": tuple(eb), "NBLK": NBLK, "NSLOT": NSLOT, "FT": NBLK * FB,
           "core_of": core_of, "slot_of": slot_of, "gidx": gidx,
           "nodes_by_core": nodes_by_core, "slots_by_core": slots_by_core}
    _LAYOUT[key] = lay
    return lay


def _get_rt(eb):
    """Compile the Bass program and build the cached jitted dispatch."""
    key = tuple(eb)
    if key in _RT:
        return _RT[key]
    import jax
    from jax.sharding import Mesh, PartitionSpec, NamedSharding
    from jax.experimental.shard_map import shard_map
    from concourse import mybir
    from concourse.bass2jax import (_bass_exec_p, install_neuronx_cc_hook,
                                    partition_id_tensor)

    install_neuronx_cc_hook()
    nc = _build_nc(list(key))

    partition_name = nc.partition_id_tensor.name if nc.partition_id_tensor else None
    in_names, out_names, out_avals, zero_shapes = [], [], [], []
    for alloc in nc.m.functions[0].allocations:
        if not isinstance(alloc, mybir.MemoryLocationSet):
            continue
        name = alloc.memorylocations[0].name
        if alloc.kind == "ExternalInput":
            if name != partition_name:
                in_names.append(name)
        elif alloc.kind == "ExternalOutput":
            out_names.append(name)
            shape = tuple(alloc.tensor_shape)
            dtype = mybir.dt.np(alloc.dtype)
            out_avals.append(jax.core.ShapedArray(shape, dtype))
            zero_shapes.append((shape, dtype))
    n_params = len(in_names)
    in_names_full = in_names + out_names + (
        [partition_name] if partition_name else [])

    def _body(*args):
        operands = list(args)
        if partition_name is not None:
            operands.append(partition_id_tensor())
        outs = _bass_exec_p.bind(
            *operands, out_avals=tuple(out_avals),
            in_names=tuple(in_names_full), out_names=tuple(out_names),
            lowering_input_output_aliases=(), sim_require_finite=True,
            sim_require_nnan=True, nc=nc)
        return tuple(outs)

    devices = jax.devices()[:NCORES]
    mesh = Mesh(np.asarray(devices), ("core",))
    nin = n_params + len(out_names)
    sh = NamedSharding(mesh, PartitionSpec("core"))

    def _make_jit():
        return jax.jit(
            shard_map(_body, mesh=mesh,
                      in_specs=(PartitionSpec("core"),) * nin,
                      out_specs=(PartitionSpec("core"),) * len(out_names),
                      check_rep=False),
            keep_unused=True)

    # abstract avals for AOT lowering (global shapes, sharded on axis 0)
    name2shape = {}
    for alloc in nc.m.functions[0].allocations:
        if isinstance(alloc, mybir.MemoryLocationSet) and alloc.tensor_shape:
            from concourse import mybir as _mb
            name2shape[alloc.memorylocations[0].name] = (
                tuple(alloc.tensor_shape), _mb.dt.np(alloc.dtype))
    structs = []
    for nm in in_names + out_names:
        shp, dt = name2shape[nm]
        structs.append(jax.ShapeDtypeStruct(
            (NCORES * shp[0],) + tuple(shp[1:]), dt, sharding=sh))
    try:
        from concourse.bass2jax import fast_dispatch_compile
        sharded = fast_dispatch_compile(
            lambda: _make_jit().lower(*structs).compile())
    except Exception:
        sharded = _make_jit()
    # zero output operands live on device; not donated, so reusable forever
    zeros = [jax.device_put(
        np.zeros((NCORES * s[0], *s[1:]), dt), sh) for s, dt in zero_shapes]
    rt = {"nc": nc, "sharded": sharded, "in_names": in_names,
          "out_names": out_names, "zeros": zeros, "sh": sh, "mesh": mesh}
    _RT[key] = rt
    return rt


def _get_dev_consts(inputs, rt):
    import jax
    import hashlib
    h = hashlib.md5()
    for k in ("U3_0", "U2_0", "U1_0", "W3_0", "W2_0", "W1_0",
              "U3_1", "U2_1", "U1_1", "W3_1", "W2_1", "W1_1"):
        h.update(np.ascontiguousarray(np.asarray(inputs[k])).tobytes())
    key = h.hexdigest()
    if key in _DEV_CONSTS:
        return _DEV_CONSTS[key]
    consts = _build_consts(inputs)
    dev = {k: jax.device_put(np.tile(v, (NCORES,) + (1,) * (v.ndim - 1)),
                             rt["sh"]) for k, v in consts.items()}
    for z in dev.values():
        z.block_until_ready()
    _DEV_CONSTS[key] = dev
    return dev


def _run(inputs):
    x = np.asarray(inputs["node_feats"], np.float32)
    sc = np.asarray(inputs["sc"], np.float32)
    y = np.asarray(inputs["node_attrs"], np.float32)
    Wlin0 = np.asarray(inputs["Wlin0"], np.float32)
    Wlin1 = np.asarray(inputs["Wlin1"], np.float32)

    elem = np.argmax(y, axis=1)
    lay = _layout(elem)
    NSLOT = lay["NSLOT"]
    rt = _get_rt(lay["eb"])
    dev = _get_dev_consts(inputs, rt)

    # one fused scatter: xt4d[core, i, slot, c] = x[node, c, i]; pad slots
    # stay zero from allocation (node slots are overwritten every call)
    xt4d = lay.setdefault(
        "xt4d", np.zeros((NCORES, I, NSLOT, C), np.float16))
    xt4d[lay["core_of"], :, lay["slot_of"]] = x.swapaxes(1, 2)
    xt16 = xt4d.reshape(NCORES * I, lay["FT"])

    args = [xt16 if nm == "XT" else dev[nm] for nm in rt["in_names"]]
    out = rt["sharded"](*args, *rt["zeros"])

    # stream the fetch: post-process each core's nodes as its shard lands,
    # so the Linear/permute work hides behind the remaining downloads
    out_np = np.empty((N, C * 4), np.float32)

    def _post_shard(s):
        fsh = np.asarray(s.data)                  # [4, FT] f16, blocks here
        c = s.index[0].start // 4
        nodes = lay["nodes_by_core"][c]
        fg = fsh.reshape(4, NSLOT, C)[
            :, lay["slots_by_core"][c], :].astype(np.float32)
        nn = len(nodes)
        out_np[nodes, :C] = fg[0] @ Wlin0
        z = (fg[1:].reshape(3 * nn, C) @ Wlin1).reshape(3, nn, C)
        out_np[nodes, C:] = z.transpose(1, 2, 0).reshape(nn, 3 * C)

    list(_pool().map(_post_shard, out[0].addressable_shards))

    out_np *= np.float32(1.0 / np.sqrt(C))
    out_np += sc
    return out_np


_EXEC = None


def _pool():
    global _EXEC
    if _EXEC is None:
        from concurrent.futures import ThreadPoolExecutor
        _EXEC = ThreadPoolExecutor(max_workers=NCORES)
    return _EXEC


def kernel(**inputs):
    res = _run(inputs)
    import os
    nrep = int(os.environ.get("KERNEL_TIME_RUNS", "0"))
    if nrep:
        import time
        times = []
        for _ in range(nrep):
            t0 = time.perf_counter()
            _run(inputs)
            times.append(time.perf_counter() - t0)
        globals()["LAST_TIMES"] = times
    return res
